# revision 19
# baseline (speedup 1.0000x reference)
"""Trainium2 Bass kernel for nn_Block1_87144886436577 (vq_codebook).

v4: on-chip data-movement redesign (no big DRAM broadcasts).

- e9 candidate table: ep patches are column-shuffled on-chip into
  dy-triple rows (ep3r), scattered once per (b, r) into a single-copy
  DRAM buffer edR3 (138KB), and loaded back as per-partition 3-row
  windows with the h-shift folded into the load base address; the
  final gather uses one group-uniform index table.
- sel masks: sel image lives in a 12KB DRAM buffer; 20 strided DMAs
  build sel_patch[(b,o), 10x10] directly; one is_equal + one PE
  transpose gives S_cmp[(cy,cx), (b,o)]; the per-tap masks are 8
  constant permutation matmuls (emtX) in bf16.
- hopfield: no max-subtraction (logits are +-1), attention computed
  directly in [k, pix] orientation, V|ones matmul yields sums for
  free, normalization broadcast via a K=1 matmul then a wide
  reciprocal. Output hopfield + all of phase C run in bf16.
- g1 collapsed 16->4 matmuls (lhsT = PE-transposed w2s4 blocks),
  z collapsed 16->8 (block-diagonal w1 lhsT).

Single-core program; all 8 cores run identical replicas. Output read
from core 0.
"""
import sys

import numpy as np

for _p in ("/opt/trn_rl_repo",):
    if _p not in sys.path:
        sys.path.insert(0, _p)

import concourse.bass as bass
import concourse.mybir as mybir
import concourse.tile as tile

F32 = mybir.dt.float32
BF16 = mybir.dt.bfloat16
U16 = mybir.dt.uint16
AF = mybir.ActivationFunctionType
ALU = mybir.AluOpType
AX = mybir.AxisListType
AP = bass.AP

N_CORES = 8


def v(t, off, pat):
    return AP(t.tensor, t.offset + off, pat)


def _e(r):
    return 1 if r >= 1 else 0


def _consts():
    """Host-precomputed constant tensors (input-independent)."""
    import ml_dtypes

    ident128 = np.eye(128, dtype=np.float32)

    # x gather idx into the parity-subsampled data_x2 row
    # (row = [b0-sub 722 | b1-sub 722], sub = 38 rows x 19 even cols;
    # partition-in-group = (k1x, ci) bakes parity+shift).
    idxX = np.zeros((8, 128, 8), np.uint16)
    for t in range(8):
        k2y, k2xh = t // 2, t % 2
        for g in range(8):
            k2xp = g // 4
            k1y = g % 4
            k2x = 2 * k2xh + k2xp
            for j in range(128):
                b, oy, ox = j // 64, (j % 64) // 8, j % 8
                idxX[t, 16 * g + j % 16, j // 16] = (
                    b * 722 + (4 * oy + 2 * k2y + k1y) * 19
                    + 2 * ox + k2x)
    idxXs = np.ascontiguousarray(
        idxX.transpose(1, 0, 2).reshape(128, 64))

    # e9 gather idx into the 1296-wide data_e2 row: group-uniform.
    idxE3 = np.zeros((128, 9), np.uint16)
    for j in range(144):
        ixl, jj = j // 9, j % 9
        jy, jx = jj // 3, jj % 3
        sp, t = ixl // 4, ixl % 4
        et = 1 if t >= 1 else 0
        oxrel = sp + et - jx + 2
        dx = t - 4 * et + 4 * jx + 3
        val = (2 - jy) * 432 + oxrel * 36 + jy * 12 + dx
        idxE3[j % 16:128:16, j // 16] = val

    # argmin helper tables
    oidx9 = np.full((128, 144), 3000.0, np.float32)
    zc = np.zeros((128, 16), np.float32)
    for r in range(4):
        for h in range(2):
            for b in range(2):
                for q in range(8):
                    p = b * 64 + r * 16 + h * 8 + q
                    iy = 4 * q + r
                    for ixl in range(16):
                        ix = 16 * h + ixl
                        t_ = ix % 4
                        s = ix // 4
                        for jj in range(9):
                            jy, jx = jj // 3, jj % 3
                            oy = q + _e(r) - jy
                            ox = s + _e(t_) - jx
                            dy = iy - 4 * oy + 3
                            dx = ix - 4 * ox + 3
                            if (0 <= oy < 8 and 0 <= ox < 8
                                    and 0 <= dy < 10 and 0 <= dx < 10):
                                oidx9[p, ixl * 9 + jj] = oy * 8 + ox
                        for o in range(64):
                            oy, ox = o // 8, o % 8
                            if not (0 <= iy - 4 * oy + 3 < 10
                                    and 0 <= ix - 4 * ox + 3 < 10):
                                zc[p, ixl] = float(o)
                                break

    # emtX[t]: constant permutation: S_cmp row c=(10*cy+cx) -> mask
    # row p=(k2xp, k1y, ci, k1x) for tap tile t=(k2y, k2xh).
    emX = np.zeros((8, 128, 128), np.float32)
    for t in range(8):
        k2y, k2xh = t // 2, t % 2
        for p in range(128):
            k2xp = p // 64
            k1y = (p // 16) % 4
            k1x = (p % 16) // 4
            k2x = 2 * k2xh + k2xp
            cy = 2 * k2y + k1y
            cx = 2 * k2x + k1x
            emX[t, 10 * cy + cx, p] = 1.0
    emtX = np.ascontiguousarray(
        emX.transpose(1, 0, 2).reshape(128, 1024)).astype(ml_dtypes.bfloat16)

    # per-partition output index o = p % 64 (for sel_patch compare)
    oidxP = (np.arange(128) % 64).astype(np.float32).reshape(128, 1)

    return {"ident128": ident128, "idxXs": idxXs, "idxE3": idxE3,
            "oidx9": oidx9, "zc": zc, "emtX": emtX, "oidxP": oidxP}


def build_program(nc, debug=False):
    x_d = nc.declare_dram_parameter("x", [2, 3, 32, 32], F32, isOutput=False)
    w1_d = nc.declare_dram_parameter("w1", [32, 3, 4, 4], F32, isOutput=False)
    b1_d = nc.declare_dram_parameter("b1", [32], F32, isOutput=False)
    w2_d = nc.declare_dram_parameter("w2", [64, 32, 4, 4], F32, isOutput=False)
    b2_d = nc.declare_dram_parameter("b2", [64], F32, isOutput=False)
    k_d = nc.declare_dram_parameter("K", [512, 64], F32, isOutput=False)
    v_d = nc.declare_dram_parameter("V", [512, 64], F32, isOutput=False)
    id_d = nc.declare_dram_parameter("ident128", [128, 128], F32,
                                     isOutput=False)
    ixx_d = nc.declare_dram_parameter("idxXs", [128, 64], U16, isOutput=False)
    ixe_d = nc.declare_dram_parameter("idxE3", [128, 9], U16, isOutput=False)
    oi9_d = nc.declare_dram_parameter("oidx9", [128, 144], F32,
                                      isOutput=False)
    zc_d = nc.declare_dram_parameter("zc", [128, 16], F32, isOutput=False)
    emx_d = nc.declare_dram_parameter("emtX", [128, 1024], BF16,
                                      isOutput=False)
    oip_d = nc.declare_dram_parameter("oidxP", [128, 1], F32, isOutput=False)
    out_d = nc.declare_dram_parameter("out", [2, 64, 8, 8], F32,
                                      isOutput=True)
    dbg = {}
    if debug:
        for nm, sh in [("d_yT", [64, 128]), ("d_r2T", [64, 128]),
                       ("d_ep2pad", [128, 144]), ("d_e9", [128, 144]),
                       ("d_sel16", [128, 16]), ("d_selpatch", [128, 100]),
                       ("d_Scmp", [128, 128]), ("d_ymmT", [64, 128])]:
            dbg[nm] = nc.declare_dram_parameter(nm, sh, F32, isOutput=True)

    with tile.TileContext(nc) as tc:
        with (
            tc.tile_pool(name="const", bufs=1) as cpool,
            tc.tile_pool(name="work", bufs=1) as wpool,
            tc.tile_pool(name="psA", bufs=2, space="PSUM") as psA,
            tc.tile_pool(name="psB", bufs=4, space="PSUM") as psB,
            tc.tile_pool(name="psC", bufs=2, space="PSUM") as psC,
            tc.tile_pool(name="dram", bufs=1, space="DRAM") as dpool,
        ):
            dma = nc.sync.dma_start
            sdma = nc.scalar.dma_start
            gdma = nc.gpsimd.dma_start

            # ---- DRAM scratch ----
            x_pad2 = dpool.tile([11680], F32)      # [ci][b][38x38] + slop
            selD = dpool.tile([3040], F32)         # [b][38x40] sel image
            edR3 = dpool.tile([35136], F32)        # [b][r][10][12][36] + pad

            # ---- critical-path staging ----
            ident = cpool.tile([128, 128], F32)
            dma(ident[:], id_d[:])
            w1sb = wpool.tile([32, 48], F32)          # [m, (ci,k1)]
            dma(w1sb[:], AP(w1_d, 0, [[48, 32], [1, 48]]))
            xp4 = wpool.tile([128, 2888], F32)
            xpp = xp4.ap[0][0]
            nc.gpsimd.memset(xp4[:], 0.0)
            for l in range(4):
                for b in range(2):
                    eng = dma if (2 * l + b) % 2 == 0 else sdma
                    eng(v(xp4, 32 * l * xpp + b * 1444 + 117 - l,
                          [[xpp, 3], [38, 32], [1, 32]]),
                        AP(x_d, b * 3072, [[1024, 3], [32, 32], [1, 32]]))

            # conv1 lhsT: one transpose of IN where
            # IN[32*k1y + m, 32*k1x + ci] = w1[m, ci, k1y, k1x]
            w1IN = wpool.tile([128, 128], F32)
            nc.gpsimd.memset(w1IN[:], 0.0)
            wip = w1IN.ap[0][0]
            for k1y in range(4):
                nc.scalar.copy(
                    v(w1IN, 32 * k1y * wip,
                      [[wip, 32], [32, 4], [1, 3]]),
                    v(w1sb, 4 * k1y, [[w1sb.ap[0][0], 32], [1, 4], [16, 3]]))
            wg_ps = psB.tile([128, 128], F32, tag="psB", name="wg_ps")
            nc.tensor.transpose(wg_ps[:], w1IN[:], ident[:])
            w1gall = wpool.tile([128, 128], F32)
            nc.scalar.copy(w1gall[:], wg_ps[:])

            b1t4 = wpool.tile([32, 1], F32)
            sdma(b1t4[:], AP(b1_d, 0, [[1, 32], [1, 1]]))
            b2t = wpool.tile([64, 1], F32)
            sdma(b2t[:], AP(b2_d, 0, [[1, 64], [1, 1]]))

            # ---- x broadcast prep (HWDGE; needed by phase C only) ----
            zx = cpool.tile([16, 2336], F32)
            nc.gpsimd.memset(zx[:], 0.0)
            sdma(v(x_pad2, 0, [[2336, 5], [1, 2336]]), zx[0:5, :])
            for b in range(2):
                (dma if b == 0 else sdma)(
                    v(x_pad2, b * 1444 + 117,
                      [[2888, 3], [38, 32], [1, 32]]),
                    AP(x_d, b * 3072, [[1024, 3], [32, 32], [1, 32]]))
            xpads = wpool.tile([4, 2888], F32)
            sdma(xpads[:], v(x_pad2, 0, [[2888, 4], [1, 2888]]))
            xsub = wpool.tile([4, 2896], BF16)
            nc.gpsimd.memset(xsub[:], 0.0)
            xsp = xsub.ap[0][0]
            xpp2 = xpads.ap[0][0]
            for par in range(2):
                nc.vector.tensor_copy(
                    v(xsub, 1448 * par,
                      [[xsp, 3], [722, 2], [19, 38], [1, 19]]),
                    v(xpads, par,
                      [[xpp2, 3], [1444, 2], [38, 38], [2, 19]]))
            # stage the 16 distinct rows in DRAM, then 8 independent
            # re-reads broadcast them to all 128 partitions (no RAW
            # doubling chain).
            xsubD = dpool.tile([23104], BF16)      # [16 = (k1x,ci)][1444]
            for k1x in range(4):
                eng = dma if k1x % 2 == 0 else sdma
                eng(v(xsubD, k1x * 4 * 1444, [[1444, 4], [1, 1444]]),
                    v(xsub, 1448 * (k1x % 2) + (k1x // 2),
                      [[xsp, 4], [1, 1444]]))
            data_x = wpool.tile([128, 1444], BF16)
            for grep in range(8):
                eng = (dma, sdma, gdma)[grep % 3]
                eng(data_x[16 * grep:16 * (grep + 1), :],
                    v(xsubD, 0, [[1444, 16], [1, 1444]]))

            ixX = cpool.tile([128, 64], U16)
            sdma(ixX[:], ixx_d[:])
            # x gather: one batched indirect copy for all 8 tap tiles
            xgALL = wpool.tile([128, 1024], BF16)
            nc.gpsimd.indirect_copy(
                v(xgALL, 0, [[xgALL.ap[0][0], 128], [1, 1024], [1, 1]]),
                data_x[:], ixX[:], True)

            # ---- consts (scalar queue) ----
            ixE = cpool.tile([128, 9], U16)
            sdma(ixE[:], ixe_d[:])
            oidx9 = cpool.tile([128, 144], F32)
            sdma(oidx9[:], oi9_d[:])
            zct = cpool.tile([128, 16], F32)
            sdma(zct[:], zc_d[:])
            emtX = cpool.tile([128, 1024], BF16)
            sdma(emtX[:], emx_d[:])
            oidxP = cpool.tile([128, 1], F32)
            sdma(oidxP[:], oip_d[:])

            # zero fills
            zf = cpool.tile([128, 540], F32)
            nc.gpsimd.memset(zf[:], 0.0)
            gdma(v(edR3, 0, [[540, 65], [1, 540]]), zf[0:65, :])
            selDf = cpool.tile([2, 1520], F32)
            nc.gpsimd.memset(selDf[:], -1.0)
            gdma(v(selD, 0, [[1520, 2], [1, 1520]]), selDf[:])

            # conv2 weights: w2sb [m, (c,k2)] then w2s4 [(l,m), (g,c)]
            w2sb = wpool.tile([32, 1024], F32)
            sdma(w2sb[:], AP(w2_d, 0, [[16, 32], [512, 64], [1, 16]]))
            w2s4 = wpool.tile([128, 256], F32)
            for l in range(4):
                nc.vector.tensor_copy(
                    v(w2s4, 32 * l * w2s4.ap[0][0],
                      [[w2s4.ap[0][0], 32], [64, 4], [1, 64]]),
                    v(w2sb, l, [[w2sb.ap[0][0], 32], [4, 4], [16, 64]]))

            # K^T (scaled by beta) and V|ones in SBUF
            k4 = wpool.tile([128, 256], F32)
            sdma(k4[:], AP(k_d, 0, [[64, 128], [8192, 4], [1, 64]]))
            kt_sb = wpool.tile([64, 512], F32)
            for t in range(4):
                kt_ps = psB.tile([64, 128], F32, tag="psB", name=f"ktp{t}")
                nc.tensor.transpose(kt_ps[:], k4[:, 64 * t:64 * (t + 1)],
                                    ident[:])
                nc.vector.tensor_scalar(kt_sb[:, 128 * t:128 * (t + 1)],
                                        kt_ps[:], 0.125, None, ALU.mult)
            v1_sb = wpool.tile([128, 260], F32)
            sdma(v(v1_sb, 0, [[v1_sb.ap[0][0], 128], [65, 4], [1, 64]]),
                 AP(v_d, 0, [[64, 128], [8192, 4], [1, 64]]))
            nc.vector.memset(
                v(v1_sb, 64, [[v1_sb.ap[0][0], 128], [65, 4], [1, 1]]), 1.0)
            ones64 = cpool.tile([1, 64], F32)
            nc.vector.memset(ones64[:], 1.0)

            # misc pre-zeroed tiles
            ep2pad = wpool.tile([128, 144], F32)
            nc.gpsimd.memset(ep2pad[:], 0.0)
            S_cmp = wpool.tile([128, 128], BF16)
            nc.gpsimd.memset(S_cmp[:], 0.0)
            w1diag = wpool.tile([128, 64], BF16)
            nc.gpsimd.memset(w1diag[:], 0.0)

            # w1s taps for the e-patch matmuls
            w1s16 = wpool.tile([32, 16], F32)
            nc.vector.tensor_reduce(
                w1s16[:],
                v(w1sb, 0, [[w1sb.ap[0][0], 32], [1, 16], [16, 3]]),
                AX.X, ALU.add)
            w1si = wpool.tile([32, 256], F32)
            nc.gpsimd.memset(w1si[:], 0.0)
            nc.vector.tensor_copy(
                v(w1si, 102, [[w1si.ap[0][0], 32], [16, 4], [1, 4]]),
                w1s16[:])
            w1stp4 = []
            for g in range(4):
                wt = wpool.tile([128, 100], F32, name=f"w1stp4{g}")
                for l in range(4):
                    nc.vector.tensor_copy(
                        v(wt, 32 * l * wt.ap[0][0],
                          [[wt.ap[0][0], 32], [10, 10], [1, 10]]),
                        v(w1si, (6 - 2 * g) * 16 + 6 - 2 * l,
                          [[w1si.ap[0][0], 32], [16, 10], [1, 10]]))
                w1stp4.append(wt)

            # ---- Phase A: forward ----
            y1ps = psA.tile([32, 512], F32, tag="psA", name="y1ps")
            for g in range(4):
                nc.tensor.matmul(
                    y1ps[:], w1gall[:, 32 * g:32 * (g + 1)],
                    v(xp4, 78 + 38 * g, [[xpp, 128], [1444, 2],
                                         [76, 16], [2, 16]]),
                    start=(g == 0), stop=(g == 3))
            y1p4 = wpool.tile([128, 648], F32)
            nc.gpsimd.memset(y1p4[:], 0.0)
            ypitch = y1p4.ap[0][0]
            nc.scalar.activation(
                v(y1p4, 19, [[ypitch, 32], [324, 2], [18, 16], [1, 16]]),
                y1ps[:], AF.Relu, bias=b1t4[:])
            iview = [[ypitch, 32], [324, 2], [18, 16], [1, 16]]
            for l in range(1, 4):
                nc.vector.tensor_copy(
                    v(y1p4, 32 * l * ypitch + 19 - l, iview),
                    v(y1p4, 19, iview))
            m1p4 = wpool.tile([128, 648], F32)
            nc.vector.tensor_scalar(m1p4[:], y1p4[:], 0.0, None, ALU.is_gt)

            def tapg(tl, g, pitch, np_=128, base=0):
                return v(tl, base * pitch + 18 * g,
                         [[pitch, np_], [324, 2], [36, 8], [2, 8]])

            ypre = psA.tile([64, 128], F32, tag="psA", name="ypre")
            for g in range(4):
                nc.tensor.matmul(
                    ypre[:],
                    v(w2s4, 64 * g, [[w2s4.ap[0][0], 128], [1, 64]]),
                    tapg(y1p4, g, ypitch),
                    start=(g == 0), stop=(g == 3))
            yT = wpool.tile([64, 128], F32)    # [c, (b,o)]
            nc.scalar.activation(yT[:], ypre[:], AF.Relu, bias=b2t[:])
            m2T = wpool.tile([64, 128], F32)
            nc.vector.tensor_scalar(m2T[:], yT[:], 0.0, None, ALU.is_gt)

            # deferred staging (needed from hop1/g1/phase-C onwards)
            g1L = []
            for g in range(4):
                gt_ps = psB.tile([64, 128], F32, tag="psB", name=f"g1Lp{g}")
                nc.tensor.transpose(gt_ps[:], w2s4[:, 64 * g:64 * (g + 1)],
                                    ident[:])
                gl = wpool.tile([64, 128], F32, name=f"g1L{g}")
                nc.vector.tensor_copy(gl[:], gt_ps[:])
                g1L.append(gl)
            w2s4b = wpool.tile([128, 256], BF16)
            nc.vector.tensor_copy(w2s4b[:], w2s4[:])
            ktb = wpool.tile([64, 512], BF16)
            nc.vector.tensor_copy(ktb[:], kt_sb[:])
            v1b = wpool.tile([128, 260], BF16)
            nc.vector.tensor_copy(v1b[:], v1_sb[:])
            w1sb2 = wpool.tile([32, 48], F32)
            sdma(w1sb2[:], AP(w1_d, 0, [[48, 32], [16, 3], [1, 16]]))
            w1sb2p = wpool.tile([32, 64], F32)
            nc.gpsimd.memset(w1sb2p[:], 0.0)
            nc.scalar.copy(
                v(w1sb2p, 0, [[w1sb2p.ap[0][0], 32], [16, 4], [4, 4], [1, 3]]),
                v(w1sb2, 0, [[w1sb2.ap[0][0], 32], [4, 4], [1, 4], [16, 3]]))
            w1fp_ps = psB.tile([64, 32], F32, tag="psB", name="w1fp_ps")
            nc.tensor.transpose(w1fp_ps[:], w1sb2p[:], ident[0:32, 0:32])
            nc.vector.tensor_copy(w1diag[0:64, 0:32], w1fp_ps[:])
            nc.vector.tensor_copy(w1diag[64:128, 32:64], w1fp_ps[:])


            # ---- hopfield 1 (fp32, argmin-critical) ----
            pt1 = []
            for t in range(4):
                pT_ps = psB.tile([128, 128], F32, tag="psB", name=f"pT1{t}")
                nc.tensor.matmul(pT_ps[:], kt_sb[:, 128 * t:128 * (t + 1)],
                                 yT[:], start=True, stop=True)
                pt = wpool.tile([128, 128], F32, name=f"pt1{t}")
                nc.scalar.activation(pt[:], pT_ps[:], AF.Exp)
                pt1.append(pt)
            yq_ps = psC.tile([65, 128], F32, tag="psC", name="yq1")
            for t in range(4):
                nc.tensor.matmul(yq_ps[:],
                                 v(v1_sb, 65 * t, [[v1_sb.ap[0][0], 128],
                                                   [1, 65]]),
                                 pt1[t][:], start=(t == 0), stop=(t == 3))
            ssum1 = wpool.tile([1, 128], F32)
            nc.scalar.copy(ssum1[:], yq_ps[64:65, :])
            rb_ps = psB.tile([64, 128], F32, tag="psB", name="rb1")
            nc.tensor.matmul(rb_ps[:], ones64[:], ssum1[:], start=True,
                             stop=True)
            rbi = wpool.tile([64, 128], F32)
            nc.vector.reciprocal(rbi[:], rb_ps[:])
            r2T = wpool.tile([64, 128], F32)
            nc.vector.tensor_tensor(r2T[:], yq_ps[0:64, :], rbi[:],
                                    ALU.mult)
            nc.vector.scalar_tensor_tensor(r2T[:], r2T[:], -1.0, yT[:],
                                           ALU.mult, ALU.add)
            nc.vector.tensor_tensor(r2T[:], r2T[:], m2T[:], ALU.mult)

            # ---- Phase B: e-patch + argmin ----
            g1m4 = []
            for g in range(4):
                g1ps = psB.tile([128, 128], F32, tag="psB", name=f"g1ps{g}")
                nc.tensor.matmul(g1ps[:], g1L[g][:], r2T[:], start=True,
                                 stop=True)
                gm = wpool.tile([128, 128], F32, name=f"g1m4{g}")
                nc.vector.tensor_tensor(gm[:], g1ps[:],
                                        tapg(m1p4, g, ypitch), ALU.mult)
                g1m4.append(gm)

            ep_ps = psA.tile([100, 128], F32, tag="psA", name="ep_ps")
            for g in range(4):
                nc.tensor.matmul(ep_ps[:], w1stp4[g][:], g1m4[g][:],
                                 start=(g == 0), stop=(g == 3))
            ep_sb = wpool.tile([100, 128], F32)
            nc.scalar.copy(ep_sb[:], ep_ps[:])
            ep2_ps = psB.tile([128, 100], F32, tag="psB", name="ep2_ps")
            nc.tensor.transpose(ep2_ps[:], ep_sb[:], ident[0:100, 0:100])
            nc.scalar.copy(
                v(ep2pad, 0, [[ep2pad.ap[0][0], 128], [12, 10], [1, 10]]),
                ep2_ps[:])

            # dy-triple shuffle + single-copy scatter + windowed load
            data_e2 = wpool.tile([128, 1296], F32)
            for r in range(4):
                dy0 = r - 4 * _e(r) + 3
                e3 = wpool.tile([128, 36], F32, name=f"ep3r{r}")
                nc.vector.tensor_copy(
                    e3[:],
                    v(ep2pad, dy0 * 12,
                      [[ep2pad.ap[0][0], 128], [48, 3], [1, 12]]))
                for b in range(2):
                    base = (b * 17280 + r * 4320 + (2 - _e(r)) * 432
                            + 2 * 36)
                    eng = dma if b == 0 else sdma
                    eng(v(edR3, base, [[432, 8], [36, 8], [1, 36]]),
                        e3[b * 64:(b + 1) * 64, :])
            for r in range(4):
                for b in range(2):
                    eng = (dma, sdma, gdma)[(2 * r + b) % 3]
                    eng(data_e2[b * 64 + 16 * r:b * 64 + 16 * (r + 1), :],
                        v(edR3, b * 17280 + r * 4320,
                          [[144, 2], [432, 8], [1, 1296]]))

            e9 = wpool.tile([128, 144], F32)
            e9pitch = e9.ap[0][0]
            nc.gpsimd.indirect_copy(
                v(e9, 0, [[e9pitch, 128], [1, 144], [1, 1]]),
                data_e2[:], ixE[:], True)

            # argmin with reference tie semantics
            mincand = wpool.tile([128, 16], F32)
            nc.vector.tensor_reduce(
                mincand[:], v(e9, 0, [[e9pitch, 128], [9, 16], [1, 9]]),
                AX.X, ALU.min)
            mstar = wpool.tile([128, 16], F32)
            nc.vector.tensor_scalar(mstar[:], mincand[:], 0.0, None, ALU.min)
            eq9 = wpool.tile([128, 144], F32)
            nc.vector.tensor_tensor(
                v(eq9, 0, [[eq9.ap[0][0], 128], [9, 16], [1, 9]]),
                v(e9, 0, [[e9pitch, 128], [9, 16], [1, 9]]),
                v(mstar, 0, [[mstar.ap[0][0], 128], [1, 16], [0, 9]]),
                ALU.is_equal)
            cs = wpool.tile([128, 144], F32)
            nc.vector.scalar_tensor_tensor(cs[:], eq9[:], -1000.0, oidx9[:],
                                           ALU.mult, ALU.add)
            minc2 = wpool.tile([128, 16], F32)
            nc.vector.tensor_reduce(
                minc2[:], v(cs, 0, [[cs.ap[0][0], 128], [9, 16], [1, 9]]),
                AX.X, ALU.min)
            zeq = wpool.tile([128, 16], F32)
            nc.vector.tensor_scalar(zeq[:], mstar[:], 0.0, None,
                                    ALU.is_equal)
            zsc = wpool.tile([128, 16], F32)
            nc.vector.scalar_tensor_tensor(zsc[:], zeq[:], -1000.0, zct[:],
                                           ALU.mult, ALU.add)
            sel16 = wpool.tile([128, 16], F32)
            nc.vector.tensor_tensor(sel16[:], minc2[:], zsc[:], ALU.min)
            nc.vector.tensor_scalar(sel16[:], sel16[:], 1000.0, None,
                                    ALU.add)

            # ---- sel -> masks via patch trick ----
            nsc = 0
            for b in range(2):
                for r in range(4):
                    eng = (dma, sdma)[nsc % 2]
                    nsc += 1
                    eng(v(selD, b * 1520 + (r + 3) * 40 + 3,
                          [[16, 2], [160, 8], [1, 16]]),
                        sel16[b * 64 + 16 * r:b * 64 + 16 * (r + 1), :])
            sel_patch = wpool.tile([128, 100], F32)
            spp = sel_patch.ap[0][0]
            npd = 0
            for b in range(2):
                for dy in range(10):
                    eng = (dma, sdma, gdma)[npd % 3]
                    npd += 1
                    eng(v(sel_patch, 64 * b * spp + dy * 10,
                          [[spp, 64], [1, 10]]),
                        v(selD, b * 1520 + dy * 40,
                          [[160, 8], [4, 8], [1, 10]]))
            Cf = wpool.tile([128, 100], F32)
            nc.vector.tensor_tensor(
                Cf[:], sel_patch[:],
                v(oidxP, 0, [[oidxP.ap[0][0], 128], [0, 100]]),
                ALU.is_equal)
            ct_ps = psB.tile([100, 128], F32, tag="psB", name="ct_ps")
            nc.tensor.transpose(ct_ps[:], Cf[:], ident[:])
            nc.scalar.copy(S_cmp[0:100, :], ct_ps[:])

            # ---- Phase C: masks, z, ym (bf16) ----
            zm4 = [wpool.tile([128, 128], BF16, name=f"zm4{g}")
                   for g in range(4)]
            for t in range(8):
                g, k2xh = t // 2, t % 2
                mx_ps = psB.tile([128, 128], F32, tag="psB", name=f"mx{t}")
                nc.tensor.matmul(mx_ps[:], emtX[:, 128 * t:128 * (t + 1)],
                                 S_cmp[:], start=True, stop=True)
                xs = wpool.tile([128, 128], BF16, name=f"xs{t}")
                nc.vector.tensor_tensor(xs[:], xgALL[:, 128 * t:128 * (t + 1)],
                                        mx_ps[:], ALU.mult)
                zps = psC.tile([64, 128], F32, tag="psC", name=f"zps{t}")
                nc.tensor.matmul(zps[:], w1diag[:], xs[:], start=True,
                                 stop=True)
                nc.vector.tensor_tensor(
                    zm4[g][64 * k2xh:64 * (k2xh + 1), :], zps[:],
                    tapg(m1p4, g, ypitch, np_=64, base=32 * (2 * k2xh)),
                    ALU.mult)

            ymT_ps = psA.tile([64, 128], F32, tag="psA", name="ymT_ps")
            for g in range(4):
                nc.tensor.matmul(
                    ymT_ps[:],
                    v(w2s4b, 64 * g, [[w2s4b.ap[0][0], 128], [1, 64]]),
                    zm4[g][:], start=(g == 0), stop=(g == 3))
            ymmT = wpool.tile([64, 128], BF16)
            nc.vector.tensor_tensor(ymmT[:], ymT_ps[:], m2T[:], ALU.mult)

            # ---- hopfield 2 (bf16) ----
            pt2 = []
            for t in range(4):
                pT_ps = psB.tile([128, 128], F32, tag="psB", name=f"pT2{t}")
                nc.tensor.matmul(pT_ps[:], ktb[:, 128 * t:128 * (t + 1)],
                                 ymmT[:], start=True, stop=True)
                pt = wpool.tile([128, 128], BF16, name=f"pt2{t}")
                nc.scalar.activation(pt[:], pT_ps[:], AF.Exp)
                pt2.append(pt)
            yq2_ps = psC.tile([65, 128], F32, tag="psC", name="yq2")
            for t in range(4):
                nc.tensor.matmul(yq2_ps[:],
                                 v(v1b, 65 * t, [[v1b.ap[0][0], 128],
                                                 [1, 65]]),
                                 pt2[t][:], start=(t == 0), stop=(t == 3))
            ssum2 = wpool.tile([1, 128], F32)
            nc.scalar.copy(ssum2[:], yq2_ps[64:65, :])
            rb2_ps = psB.tile([64, 128], F32, tag="psB", name="rb2")
            nc.tensor.matmul(rb2_ps[:], ones64[:], ssum2[:], start=True,
                             stop=True)
            rbi2 = wpool.tile([64, 128], F32)
            nc.vector.reciprocal(rbi2[:], rb2_ps[:])
            outT = wpool.tile([64, 128], F32)
            nc.vector.tensor_tensor(outT[:], yq2_ps[0:64, :], rbi2[:],
                                    ALU.mult)
            for b in range(2):
                eng = dma if b == 0 else sdma
                eng(AP(out_d, b * 4096, [[64, 64], [8, 8], [1, 8]]),
                    outT[:, b * 64:(b + 1) * 64])

            if debug:
                def dump(nm, t, rows, cols, cast=None):
                    if cast is not None:
                        tf = wpool.tile([rows, cols], F32, name=nm + "c")
                        nc.vector.tensor_copy(tf[:], t)
                        t = tf[:]
                    sdma(AP(dbg[nm], 0, [[cols, rows], [1, cols]]), t)
                dump("d_yT", yT[:], 64, 128)
                dump("d_r2T", r2T[:], 64, 128)
                dump("d_ep2pad", ep2pad[:], 128, 144)
                dump("d_e9", e9[:], 128, 144)
                dump("d_sel16", sel16[:], 128, 16)
                dump("d_selpatch", sel_patch[:], 128, 100)
                dump("d_Scmp", S_cmp[:], 128, 128, cast=1)
                dump("d_ymmT", ymmT[:], 64, 128, cast=1)

    return nc


_CACHE = {}


def kernel(**inputs) -> np.ndarray:
    from concourse.bass_utils import run_bass_kernel_spmd
    if "nc" not in _CACHE:
        from concourse import bacc
        nc = bacc.Bacc("TRN2", target_bir_lowering=False, debug=False,
                       num_devices=N_CORES)
        build_program(nc)
        nc.compile()
        _CACHE["nc"] = nc
        _CACHE["consts"] = _consts()
    nc = _CACHE["nc"]
    feed = {k: np.ascontiguousarray(np.asarray(val, np.float32))
            for k, val in inputs.items()}
    for k, val in _CACHE["consts"].items():
        feed[k] = val
    in_maps = [dict(feed) for _ in range(N_CORES)]
    res = run_bass_kernel_spmd(nc, in_maps, list(range(N_CORES)))
    return np.asarray(res.results[0]["out"], np.float32)
